# revision 1
# baseline (speedup 1.0000x reference)
"""Trainium2 Bass kernel for nn_Lorec (moe_routing LoRA-with-soft-routing).

Computation (per batch b):
  gate_b = softmax(MLP(LayerNorm(ctr[b])))                    [16]
  A_b[i,r] = sum_r' Wa[r*4096+i, r'] gate_b[r']               [4096,16]
  B_b[r,o] = sum_r' Wb[r*4096+o, r'] gate_b[r']               [16,4096]
  out[b] = (x[b] @ A_b) @ B_b * 2.0                           [2048,4096]

Sharding: data-parallel over bs=8 across 8 NeuronCores (one batch per core).
Gating is replicated on every core (tiny); each core selects its own batch's
gate row via a per-core one-hot input. Adapter weights replicated.

Device dataflow per core:
  - gating MLP + softmax on DVE/ACT with tiny PE transposes
  - A/B generated on PE via the Kronecker trick: G = (I_16 kron gate) [256,16],
    A-chunk = WaP^T @ G (WaP = host-relaid Wa [256,4096]), B = G^T @ WbP.
  - mm1: xaT[16,512s] += A_c^T @ xT_c over 32 i-chunks (f32r, full rate)
    where xT_c tiles come from PE transpose-mode matmuls of natural x tiles.
  - mm2: out[128s,512o] = xaT_t^T @ B (f32r), ACT/DVE copy to SBUF, DMA out.
  - SCALING(2.0) folded into Wb on host.
"""

import os
import sys

sys.path.insert(0, "/opt/trn_rl_repo")

import numpy as np

BS = 8
SEQ = 2048
IN = 4096
OUT = 4096
R = 16
CTR_OUT = 256
CTR_HID = 60
FD = 16  # FINAL_DIM
LN_EPS = 1e-5
SCALING = 2.0

P = 128
NSB = 4  # s-blocks per core
SBW = 512  # s-block width
NC_I = IN // P  # 32 i-chunks
NOB = OUT // 512  # 8 o-blocks

_COMPILED = None


def build_program(transpose_f32r=True):
    import concourse.bass as bass
    import concourse.mybir as mybir
    from concourse import bacc
    from concourse.masks import make_identity
    from concourse.tile import TileContext

    f32 = mybir.dt.float32
    f32r = mybir.dt.float32r
    AX = mybir.AxisListType.X
    ALU = mybir.AluOpType
    ACTF = mybir.ActivationFunctionType

    nc = bacc.Bacc("TRN2", target_bir_lowering=False, debug=False, num_devices=BS)

    x_d = nc.dram_tensor("x", [SEQ, IN], f32, kind="ExternalInput").ap()
    ctr_d = nc.dram_tensor("ctr", [BS, CTR_OUT], f32, kind="ExternalInput").ap()
    gam_d = nc.dram_tensor("gam", [BS, CTR_OUT], f32, kind="ExternalInput").ap()
    bet_d = nc.dram_tensor("bet", [BS, CTR_OUT], f32, kind="ExternalInput").ap()
    w1t_d = nc.dram_tensor("w1t", [P, 2 * CTR_HID], f32, kind="ExternalInput").ap()
    b1_d = nc.dram_tensor("b1", [CTR_HID, 1], f32, kind="ExternalInput").ap()
    w2t_d = nc.dram_tensor("w2t", [CTR_HID, FD], f32, kind="ExternalInput").ap()
    b2_d = nc.dram_tensor("b2", [FD, 1], f32, kind="ExternalInput").ap()
    wap_d = nc.dram_tensor("wap", [P, 2 * IN], f32r, kind="ExternalInput").ap()
    wbp_d = nc.dram_tensor("wbp", [P, 2 * OUT], f32r, kind="ExternalInput").ap()
    sel_d = nc.dram_tensor("sel", [R, BS], f32, kind="ExternalInput").ap()
    gz_d = nc.dram_tensor("gz", [P, 4 * FD], f32r, kind="ExternalInput").ap()
    y_d = nc.dram_tensor("y", [SEQ, OUT], f32, kind="ExternalOutput").ap()

    t_dt = f32r if transpose_f32r else f32

    with TileContext(nc) as tc:
        with (
            tc.tile_pool(name="const", bufs=1) as const,
            tc.tile_pool(name="gp", bufs=1) as gp,
            tc.tile_pool(name="wstream", bufs=4) as wstream,
            tc.tile_pool(name="xpool", bufs=20) as xpool,
            tc.tile_pool(name="xtpool", bufs=3) as xtpool,
            tc.tile_pool(name="xapool", bufs=2) as xapool,
            tc.tile_pool(name="opool", bufs=3) as opool,
            tc.tile_pool(name="pst_pool", bufs=2, space="PSUM") as pst_pool,
            tc.tile_pool(name="psxa_pool", bufs=1, space="PSUM") as psxa_pool,
            tc.tile_pool(name="pso_pool", bufs=3, space="PSUM") as pso_pool,
            tc.tile_pool(name="psg_pool", bufs=1, space="PSUM") as psg_pool,
        ):
            ident = const.tile([P, P], f32)
            make_identity(nc, ident)

            # ---- gating inputs ----
            ctr = gp.tile([BS, CTR_OUT], f32)
            gam = gp.tile([BS, CTR_OUT], f32)
            bet = gp.tile([BS, CTR_OUT], f32)
            w1t = gp.tile([P, 2 * CTR_HID], f32)
            b1 = gp.tile([CTR_HID, 1], f32)
            w2t = gp.tile([CTR_HID, FD], f32)
            b2 = gp.tile([FD, 1], f32)
            sel = gp.tile([R, BS], f32)
            for t, d in [
                (ctr, ctr_d), (gam, gam_d), (bet, bet_d), (w1t, w1t_d),
                (b1, b1_d), (w2t, w2t_d), (b2, b2_d), (sel, sel_d),
            ]:
                nc.gpsimd.dma_start(out=t[:], in_=d[:])

            # ---- LayerNorm on [8, 256] ----
            mean = gp.tile([BS, 1], f32)
            xc = gp.tile([BS, CTR_OUT], f32)
            sq = gp.tile([BS, CTR_OUT], f32)
            vs = gp.tile([BS, 1], f32)
            std = gp.tile([BS, 1], f32)
            rstd = gp.tile([BS, 1], f32)
            hh = gp.tile([BS, CTR_OUT], f32)
            nc.vector.tensor_reduce(mean[:], ctr[:], axis=AX, op=ALU.add)
            nc.scalar.mul(mean[:], mean[:], 1.0 / CTR_OUT)
            nc.vector.tensor_scalar_sub(xc[:], ctr[:], mean[:])
            nc.vector.tensor_mul(sq[:], xc[:], xc[:])
            nc.vector.tensor_reduce(vs[:], sq[:], axis=AX, op=ALU.add)
            eps_t = gp.tile([BS, 1], f32)
            nc.gpsimd.memset(eps_t[:], LN_EPS)
            nc.scalar.activation(std[:], vs[:], ACTF.Sqrt, bias=eps_t[:], scale=1.0 / CTR_OUT)
            nc.vector.reciprocal(rstd[:], std[:])
            nc.vector.tensor_scalar_mul(hh[:], xc[:], rstd[:])
            nc.vector.tensor_mul(hh[:], hh[:], gam[:])
            nc.vector.tensor_add(hh[:], hh[:], bet[:])

            # ---- hT [256->2x128, 8] via PE transpose ----
            hT = gp.tile([P, 2 * BS], f32)
            for h in range(2):
                pt = psg_pool.tile([P, BS], f32, tag="psg_small")
                nc.tensor.transpose(pt[:], hh[:, h * P : (h + 1) * P], ident[0:BS, 0:BS])
                nc.scalar.copy(hT[:, h * BS : (h + 1) * BS], pt[:])

            # ---- h1T = relu(W1 @ h + b1) -> [60, 8] ----
            ph1 = psg_pool.tile([CTR_HID, BS], f32, tag="psg_small")
            for h in range(2):
                nc.tensor.matmul(
                    ph1[:], w1t[:, h * CTR_HID : (h + 1) * CTR_HID],
                    hT[:, h * BS : (h + 1) * BS], start=(h == 0), stop=(h == 1),
                )
            h1T = gp.tile([CTR_HID, BS], f32)
            nc.scalar.activation(h1T[:], ph1[:], ACTF.Relu, bias=b1[:])

            # ---- logitsT = W2 @ h1 + b2 -> [16, 8] ----
            plog = psg_pool.tile([FD, BS], f32, tag="psg_small")
            nc.tensor.matmul(plog[:], w2t[:], h1T[:], start=True, stop=True)
            logitsT = gp.tile([FD, BS], f32)
            nc.scalar.activation(logitsT[:], plog[:], ACTF.Identity, bias=b2[:])

            # ---- softmax over FD per batch: transpose to [8, 16] ----
            plg = psg_pool.tile([BS, FD], f32, tag="psg_small")
            nc.tensor.transpose(plg[:], logitsT[:], ident[0:FD, 0:FD])
            lg = gp.tile([BS, FD], f32)
            nc.scalar.copy(lg[:], plg[:])
            mx = gp.tile([BS, 1], f32)
            ex = gp.tile([BS, FD], f32)
            sm = gp.tile([BS, 1], f32)
            rsm = gp.tile([BS, 1], f32)
            gate = gp.tile([BS, FD], f32)
            nc.vector.tensor_reduce(mx[:], lg[:], axis=AX, op=ALU.max)
            nc.vector.tensor_scalar_sub(ex[:], lg[:], mx[:])
            nc.scalar.activation(ex[:], ex[:], ACTF.Exp)
            nc.vector.tensor_reduce(sm[:], ex[:], axis=AX, op=ALU.add)
            nc.vector.reciprocal(rsm[:], sm[:])
            nc.vector.tensor_scalar_mul(gate[:], ex[:], rsm[:])

            # ---- gateT [16, 8], select own batch via one-hot rows ----
            pgT = psg_pool.tile([FD, BS], f32, tag="psg_small")
            nc.tensor.transpose(pgT[:], gate[:], ident[0:BS, 0:BS])
            gateT = gp.tile([FD, BS], f32)
            nc.scalar.copy(gateT[:], pgT[:])
            gsel = gp.tile([FD, BS], f32)
            gate_b = gp.tile([FD, 1], f32)
            nc.vector.tensor_mul(gsel[:], gateT[:], sel[:])
            nc.vector.tensor_reduce(gate_b[:], gsel[:], axis=AX, op=ALU.add)

            # ---- G = I_16 kron gate_b, layout [128, 2*16] ----
            gate_br = gp.tile([FD, 1], f32r)
            nc.scalar.copy(gate_br[:], gate_b[:])
            G = gp.tile([P, 2 * FD], f32r)
            nc.gpsimd.dma_start(out=G[:], in_=gz_d[:, 0 : 2 * FD])
            for r in range(FD):
                h = r // 8
                p0 = (r % 8) * 16
                nc.gpsimd.dma_start(
                    out=G[p0 : p0 + 16, h * FD + r : h * FD + r + 1],
                    in_=gate_br[0:16, 0:1],
                )

            # ---- A-gen: A_sb[p, c*16+r] = A[c*128+p, r] ----
            # Wa streamed in [128, 1024] chunks, accumulated over h in PSUM.
            A_sb = gp.tile([P, NC_I * R], f32r)
            psA = psg_pool.tile([P, 512], f32, tag="psg_big")
            for cg in range(4):
                wts = []
                for h in range(2):
                    wt = wstream.tile([P, 1024], f32r, tag="wst")
                    nc.sync.dma_start(
                        out=wt[:],
                        in_=wap_d[:, h * IN + cg * 1024 : h * IN + (cg + 1) * 1024],
                    )
                    wts.append(wt)
                for cc in range(8):
                    c = cg * 8 + cc
                    for h in range(2):
                        nc.tensor.matmul(
                            psA[:, c * R : (c + 1) * R],
                            wts[h][:, cc * P : (cc + 1) * P],
                            G[:, h * FD : (h + 1) * FD],
                            start=(h == 0), stop=(h == 1),
                        )
            nc.scalar.copy(A_sb[:], psA[:])

            # ---- B-gen: B_sb [16, 4096] (f32r), h-accumulated in PSUM ----
            B_sb = gp.tile([FD, OUT], f32r)
            for og in range(4):
                wbs = []
                for h in range(2):
                    wt = wstream.tile([P, 1024], f32r, tag="wst")
                    nc.sync.dma_start(
                        out=wt[:],
                        in_=wbp_d[:, h * OUT + og * 1024 : h * OUT + (og + 1) * 1024],
                    )
                    wbs.append(wt)
                for oo in range(2):
                    ob = og * 2 + oo
                    psB = psg_pool.tile([FD, 512], f32, tag="psg_big")
                    for h in range(2):
                        nc.tensor.matmul(
                            psB[:],
                            G[:, h * FD : (h + 1) * FD],
                            wbs[h][:, oo * 512 : (oo + 1) * 512],
                            start=(h == 0), stop=(h == 1),
                        )
                    nc.scalar.copy(B_sb[:, ob * 512 : (ob + 1) * 512], psB[:])

            # ---- main loop over s-blocks ----
            for sb in range(NSB):
                xcts = {}
                for cg in range(4):
                    for t in range(4):
                        xt = xpool.tile([P, 1024], f32, tag="xnat")
                        nc.sync.dma_start(
                            out=xt[:],
                            in_=x_d[
                                (sb * 4 + t) * P : (sb * 4 + t + 1) * P,
                                cg * 1024 : (cg + 1) * 1024,
                            ],
                        )
                        xcts[(t, cg)] = xt

                psxa = psxa_pool.tile([FD, SBW], f32, tag="psxa")
                # software-pipelined: transposes for chunk c+1 queued before mm1(c)
                pend = None  # (xT tile, chunk)
                for c in range(NC_I):
                    psT = pst_pool.tile([P, SBW], f32, tag="pst")
                    cg, cc = c // 8, c % 8
                    for t in range(4):
                        nc.tensor.transpose(
                            psT[:, t * P : (t + 1) * P],
                            xcts[(t, cg)][:, cc * P : (cc + 1) * P],
                            ident[:],
                        )
                    xT = xtpool.tile([P, SBW], f32r, tag="xT")
                    nc.scalar.copy(xT[:], psT[:])
                    if pend is not None:
                        pxT, pc = pend
                        nc.tensor.matmul(
                            psxa[:],
                            A_sb[:, pc * R : (pc + 1) * R],
                            pxT[:],
                            start=(pc == 0), stop=False,
                        )
                    pend = (xT, c)
                pxT, pc = pend
                nc.tensor.matmul(
                    psxa[:],
                    A_sb[:, pc * R : (pc + 1) * R],
                    pxT[:],
                    start=False, stop=True,
                )

                xaT = xapool.tile([FD, SBW], f32r, tag="xaT")
                nc.scalar.copy(xaT[:], psxa[:])

                for t in range(4):
                    out_sb = opool.tile([P, OUT], f32, tag="osb")
                    for ob in range(NOB):
                        pso = pso_pool.tile([P, 512], f32, tag="pso")
                        nc.tensor.matmul(
                            pso[:],
                            xaT[:, t * P : (t + 1) * P],
                            B_sb[:, ob * 512 : (ob + 1) * 512],
                            start=True, stop=True,
                        )
                        if ob % 2 == 0:
                            nc.scalar.copy(out_sb[:, ob * 512 : (ob + 1) * 512], pso[:])
                        else:
                            nc.vector.tensor_copy(out_sb[:, ob * 512 : (ob + 1) * 512], pso[:])
                    nc.scalar.dma_start(
                        out=y_d[(sb * 4 + t) * P : (sb * 4 + t + 1) * P, :],
                        in_=out_sb[:],
                    )

    nc.compile()
    return nc


def host_prep(inputs):
    """Build per-core and shared input arrays from the full problem inputs."""
    x = np.asarray(inputs["x"], np.float32)
    ctr = np.ascontiguousarray(np.asarray(inputs["ctr_hidden_states"], np.float32))
    gam = np.ascontiguousarray(
        np.tile(np.asarray(inputs["ln_gamma"], np.float32)[None, :], (BS, 1))
    )
    bet = np.ascontiguousarray(
        np.tile(np.asarray(inputs["ln_beta"], np.float32)[None, :], (BS, 1))
    )
    W1 = np.asarray(inputs["W1"], np.float32)
    w1t = np.ascontiguousarray(
        W1.T.reshape(2, P, CTR_HID).transpose(1, 0, 2).reshape(P, 2 * CTR_HID)
    )
    b1 = np.ascontiguousarray(np.asarray(inputs["b1"], np.float32).reshape(CTR_HID, 1))
    w2t = np.ascontiguousarray(np.asarray(inputs["W2"], np.float32).T)
    b2 = np.ascontiguousarray(np.asarray(inputs["b2"], np.float32).reshape(FD, 1))
    Wa = np.asarray(inputs["Wa"], np.float32)
    WaP = Wa.reshape(R, IN, FD).transpose(0, 2, 1).reshape(R * FD, IN)
    wap = np.ascontiguousarray(
        WaP.reshape(2, P, IN).transpose(1, 0, 2).reshape(P, 2 * IN)
    )
    Wb = np.asarray(inputs["Wb"], np.float32) * SCALING
    WbP = Wb.reshape(R, OUT, FD).transpose(0, 2, 1).reshape(R * FD, OUT)
    wbp = np.ascontiguousarray(
        WbP.reshape(2, P, OUT).transpose(1, 0, 2).reshape(P, 2 * OUT)
    )

    shared = dict(
        ctr=ctr, gam=gam, bet=bet, w1t=w1t, b1=b1, w2t=w2t, b2=b2, wap=wap, wbp=wbp
    )
    in_maps = []
    for c in range(BS):
        onehot = np.zeros((BS,), np.float32)
        onehot[c] = 1.0
        sel = np.ascontiguousarray(np.tile(onehot[None, :], (R, 1)))
        m = dict(shared)
        m["sel"] = sel
        m["gz"] = np.zeros((P, 4 * FD), np.float32)
        m["x"] = np.ascontiguousarray(x[c])
        in_maps.append(m)
    return in_maps


def get_compiled():
    global _COMPILED
    if _COMPILED is None:
        _COMPILED = build_program()
    return _COMPILED


def run(inputs, trace=False):
    from concourse.bass_utils import run_bass_kernel_spmd

    nc = get_compiled()
    in_maps = host_prep(inputs)
    res = run_bass_kernel_spmd(nc, in_maps, list(range(BS)), trace=trace)
    out = np.stack([res.results[c]["y"] for c in range(BS)], axis=0)
    return out, res


def kernel(**inputs) -> np.ndarray:
    out, _ = run(inputs, trace=False)
    return out



# revision 2
# speedup vs baseline: 1.8574x; 1.8574x over previous
"""Trainium2 Bass kernel for nn_Lorec (moe_routing LoRA-with-soft-routing).

Computation (per batch b):
  gate_b = softmax(MLP(LayerNorm(ctr[b])))                    [16]
  A_b[i,r] = sum_j Wa[r*4096+i, j] gate_b[j]                  [4096,16]
  B_b[r,o] = sum_j Wb[r*4096+o, j] gate_b[j]                  [16,4096]
  out[b] = (x[b] @ A_b) @ B_b * 2.0                           [2048,4096]

Sharding: data-parallel over bs=8 across 8 NeuronCores (one batch per core).
Gating replicated on every core; each core selects its own batch's gate row
via a per-core one-hot input.

v2 (bf16): all bulk tensors are bf16 (x, Wa, Wb, A, B, xa, y) — halves HBM
traffic vs f32 (75 MB -> ~37 MB per core) and rides the full-rate bf16 PE
path. x is pre-transposed and tiled on the host into [sb*128+p, c*512+s]
layout so mm1 needs no on-device transposes (lhsT = A chunk, rhs = xT tile).
Gating scalars packed into one [128, 1074] f32 tensor -> single DMA.
G (I_16 kron gate) built arithmetically: c16[p] = gate[p%16] via one tiny
matmul with a constant selector, then G = mask * c16 — no scatter DMAs.
Output written as bf16 and upcast on host. SCALING folded into Wb on host.
"""

import sys

sys.path.insert(0, "/opt/trn_rl_repo")

import numpy as np
import ml_dtypes

BF = ml_dtypes.bfloat16

BS = 8
SEQ = 2048
IN = 4096
OUT = 4096
R = 16
CTR_OUT = 256
CTR_HID = 60
FD = 16  # FINAL_DIM
LN_EPS = 1e-5
SCALING = 2.0

P = 128
NSB = 4  # s-blocks per core
SBW = 512  # s-block width
NC_I = IN // P  # 32 i-chunks
NG = 4  # x DMA groups per s-block (8 chunks = 1 MB each)
NOB = OUT // 512  # 8 o-blocks

# gpk column layout (packed f32 gating constants; partition rows as noted)
C_W1T = 0  # [128, 120]
C_CTR = 120  # [8, 256]
C_GAM = 376  # [8, 256]
C_BET = 632  # [8, 256]
C_B1 = 888  # [60, 1]
C_W2T = 889  # [60, 16]
C_B2 = 905  # [16, 1]
C_T16 = 906  # [16, 128]  t16[j, p] = (p % 16 == j)
C_MSK = 1034  # [128, 32]  mask[p, h*16+r] = (h == r//8 and p//16 == r%8)
C_SEL = 1066  # [16, 8]   per-core one-hot columns
GPK_COLS = 1074

_COMPILED = None


def build_program():
    import concourse.mybir as mybir
    from concourse import bacc
    from concourse.masks import make_identity
    from concourse.tile import TileContext

    f32 = mybir.dt.float32
    bf16 = mybir.dt.bfloat16
    AX = mybir.AxisListType.X
    ALU = mybir.AluOpType
    ACTF = mybir.ActivationFunctionType

    nc = bacc.Bacc("TRN2", target_bir_lowering=False, debug=False, num_devices=BS)

    xtr_d = nc.dram_tensor("xtr", [NSB * P, NC_I * SBW], bf16, kind="ExternalInput").ap()
    gpk_d = nc.dram_tensor("gpk", [P, GPK_COLS], f32, kind="ExternalInput").ap()
    wap_d = nc.dram_tensor("wap", [P, 2 * IN], bf16, kind="ExternalInput").ap()
    wbp_d = nc.dram_tensor("wbp", [P, 2 * OUT], bf16, kind="ExternalInput").ap()
    y_d = nc.dram_tensor("y", [SEQ, OUT], bf16, kind="ExternalOutput").ap()

    with TileContext(nc) as tc:
        with (
            tc.tile_pool(name="const", bufs=1) as const,
            tc.tile_pool(name="gp", bufs=1) as gp,
            tc.tile_pool(name="wstream", bufs=4) as wstream,
            tc.tile_pool(name="xpool", bufs=8) as xpool,
            tc.tile_pool(name="xapool", bufs=2) as xapool,
            tc.tile_pool(name="opool", bufs=2) as opool,
            tc.tile_pool(name="psg_pool", bufs=1, space="PSUM") as psg_pool,
            tc.tile_pool(name="psA_pool", bufs=1, space="PSUM") as psA_pool,
            tc.tile_pool(name="psB_pool", bufs=2, space="PSUM") as psB_pool,
            tc.tile_pool(name="psxa_pool", bufs=1, space="PSUM") as psxa_pool,
            tc.tile_pool(name="pso_pool", bufs=3, space="PSUM") as pso_pool,
        ):
            ident = const.tile([P, P], f32)
            make_identity(nc, ident)

            # ---- all HBM loads, queued in priority order on the sync ring ----
            gpk = gp.tile([P, GPK_COLS], f32)
            nc.sync.dma_start(out=gpk[:], in_=gpk_d[:])

            wa_t = []
            for h in range(2):
                wt = wstream.tile([P, IN], bf16, tag="wst")
                nc.sync.dma_start(out=wt[:], in_=wap_d[:, h * IN : (h + 1) * IN])
                wa_t.append(wt)
            wb_t = []
            for h in range(2):
                wt = wstream.tile([P, OUT], bf16, tag="wst")
                nc.sync.dma_start(out=wt[:], in_=wbp_d[:, h * OUT : (h + 1) * OUT])
                wb_t.append(wt)

            # ---- LayerNorm on ctr [8, 256] ----
            ctr = gpk[0:BS, C_CTR : C_CTR + CTR_OUT]
            gam = gpk[0:BS, C_GAM : C_GAM + CTR_OUT]
            bet = gpk[0:BS, C_BET : C_BET + CTR_OUT]
            mean = gp.tile([BS, 1], f32)
            xc = gp.tile([BS, CTR_OUT], f32)
            sq = gp.tile([BS, CTR_OUT], f32)
            vs = gp.tile([BS, 1], f32)
            std = gp.tile([BS, 1], f32)
            rstd = gp.tile([BS, 1], f32)
            hh = gp.tile([BS, CTR_OUT], f32)
            nc.vector.tensor_reduce(mean[:], ctr, axis=AX, op=ALU.add)
            nc.scalar.mul(mean[:], mean[:], 1.0 / CTR_OUT)
            nc.vector.tensor_scalar_sub(xc[:], ctr, mean[:])
            nc.vector.tensor_mul(sq[:], xc[:], xc[:])
            nc.vector.tensor_reduce(vs[:], sq[:], axis=AX, op=ALU.add)
            eps_t = gp.tile([BS, 1], f32)
            nc.gpsimd.memset(eps_t[:], LN_EPS)
            nc.scalar.activation(std[:], vs[:], ACTF.Sqrt, bias=eps_t[:], scale=1.0 / CTR_OUT)
            nc.vector.reciprocal(rstd[:], std[:])
            nc.vector.tensor_scalar_mul(hh[:], xc[:], rstd[:])
            nc.vector.tensor_mul(hh[:], hh[:], gam)
            nc.vector.tensor_add(hh[:], hh[:], bet)

            # ---- hT [256->2x128, 8] via PE transpose ----
            hT = gp.tile([P, 2 * BS], f32)
            for h in range(2):
                pt = psg_pool.tile([P, BS], f32, tag="psg")
                nc.tensor.transpose(pt[:], hh[:, h * P : (h + 1) * P], ident[0:BS, 0:BS])
                nc.scalar.copy(hT[:, h * BS : (h + 1) * BS], pt[:])

            # ---- h1T = relu(W1 @ h + b1) -> [60, 8] ----
            w1t = gpk[:, C_W1T : C_W1T + 2 * CTR_HID]
            ph1 = psg_pool.tile([CTR_HID, BS], f32, tag="psg")
            for h in range(2):
                nc.tensor.matmul(
                    ph1[:], w1t[:, h * CTR_HID : (h + 1) * CTR_HID],
                    hT[:, h * BS : (h + 1) * BS], start=(h == 0), stop=(h == 1),
                )
            h1T = gp.tile([CTR_HID, BS], f32)
            nc.scalar.activation(h1T[:], ph1[:], ACTF.Relu, bias=gpk[0:CTR_HID, C_B1 : C_B1 + 1])

            # ---- logitsT = W2 @ h1 + b2 -> [16, 8] ----
            plog = psg_pool.tile([FD, BS], f32, tag="psg")
            nc.tensor.matmul(plog[:], gpk[0:CTR_HID, C_W2T : C_W2T + FD], h1T[:], start=True, stop=True)
            logitsT = gp.tile([FD, BS], f32)
            nc.scalar.activation(logitsT[:], plog[:], ACTF.Identity, bias=gpk[0:FD, C_B2 : C_B2 + 1])

            # ---- softmax over FD per batch: transpose to [8, 16] ----
            plg = psg_pool.tile([BS, FD], f32, tag="psg")
            nc.tensor.transpose(plg[:], logitsT[:], ident[0:FD, 0:FD])
            lg = gp.tile([BS, FD], f32)
            nc.scalar.copy(lg[:], plg[:])
            mx = gp.tile([BS, 1], f32)
            ex = gp.tile([BS, FD], f32)
            sm = gp.tile([BS, 1], f32)
            rsm = gp.tile([BS, 1], f32)
            gate = gp.tile([BS, FD], f32)
            nc.vector.tensor_reduce(mx[:], lg[:], axis=AX, op=ALU.max)
            nc.vector.tensor_scalar_sub(ex[:], lg[:], mx[:])
            nc.scalar.activation(ex[:], ex[:], ACTF.Exp)
            nc.vector.tensor_reduce(sm[:], ex[:], axis=AX, op=ALU.add)
            nc.vector.reciprocal(rsm[:], sm[:])
            nc.vector.tensor_scalar_mul(gate[:], ex[:], rsm[:])

            # ---- gateT [16, 8], select own batch via one-hot columns ----
            pgT = psg_pool.tile([FD, BS], f32, tag="psg")
            nc.tensor.transpose(pgT[:], gate[:], ident[0:BS, 0:BS])
            gateT = gp.tile([FD, BS], f32)
            nc.scalar.copy(gateT[:], pgT[:])
            gsel = gp.tile([FD, BS], f32)
            gate_b = gp.tile([FD, 1], f32)
            nc.vector.tensor_mul(gsel[:], gateT[:], gpk[0:FD, C_SEL : C_SEL + BS])
            nc.vector.tensor_reduce(gate_b[:], gsel[:], axis=AX, op=ALU.add)

            # ---- G = I_16 kron gate_b, layout [128, 2*16], bf16 ----
            # c16[p] = gate_b[p % 16] via selector matmul, then G = mask * c16
            psc16 = psg_pool.tile([P, 1], f32, tag="psg")
            nc.tensor.matmul(
                psc16[:], gpk[0:FD, C_T16 : C_T16 + P], gate_b[:], start=True, stop=True
            )
            c16 = gp.tile([P, 1], f32)
            nc.scalar.copy(c16[:], psc16[:])
            Gf = gp.tile([P, 2 * FD], f32)
            nc.vector.tensor_scalar_mul(Gf[:], gpk[:, C_MSK : C_MSK + 2 * FD], c16[:])
            G = gp.tile([P, 2 * FD], bf16)
            nc.vector.tensor_copy(G[:], Gf[:])

            # ---- A-gen: A_sb[p, c*16+r] = A[c*128+p, r], bf16 ----
            A_sb = gp.tile([P, NC_I * R], bf16)
            psA = psA_pool.tile([P, 512], f32, tag="psA")
            for c in range(NC_I):
                for h in range(2):
                    nc.tensor.matmul(
                        psA[:, c * R : (c + 1) * R],
                        wa_t[h][:, c * P : (c + 1) * P],
                        G[:, h * FD : (h + 1) * FD],
                        start=(h == 0), stop=(h == 1),
                    )
            nc.scalar.copy(A_sb[:], psA[:])

            # ---- B-gen: B_sb [16, 4096] bf16 ----
            B_sb = gp.tile([FD, OUT], bf16)
            for ob in range(NOB):
                psB = psB_pool.tile([FD, 512], f32, tag="psB")
                for h in range(2):
                    nc.tensor.matmul(
                        psB[:],
                        G[:, h * FD : (h + 1) * FD],
                        wb_t[h][:, ob * 512 : (ob + 1) * 512],
                        start=(h == 0), stop=(h == 1),
                    )
                nc.vector.tensor_copy(B_sb[:, ob * 512 : (ob + 1) * 512], psB[:])

            # ---- main loop over s-blocks ----
            for sb in range(NSB):
                xts = []
                for g in range(NG):
                    xt = xpool.tile([P, 8 * SBW], bf16, tag="xg")
                    nc.sync.dma_start(
                        out=xt[:],
                        in_=xtr_d[sb * P : (sb + 1) * P, g * 8 * SBW : (g + 1) * 8 * SBW],
                    )
                    xts.append(xt)

                psxa = psxa_pool.tile([FD, SBW], f32, tag="psxa")
                for c in range(NC_I):
                    nc.tensor.matmul(
                        psxa[:],
                        A_sb[:, c * R : (c + 1) * R],
                        xts[c // 8][:, (c % 8) * SBW : (c % 8 + 1) * SBW],
                        start=(c == 0), stop=(c == NC_I - 1),
                    )
                xaT = xapool.tile([FD, SBW], bf16, tag="xaT")
                nc.scalar.copy(xaT[:], psxa[:])

                for t in range(4):
                    out_sb = opool.tile([P, OUT], bf16, tag="osb")
                    for ob in range(NOB):
                        pso = pso_pool.tile([P, 512], f32, tag="pso")
                        nc.tensor.matmul(
                            pso[:],
                            xaT[:, t * P : (t + 1) * P],
                            B_sb[:, ob * 512 : (ob + 1) * 512],
                            start=True, stop=True,
                        )
                        if ob % 2 == 0:
                            nc.scalar.copy(out_sb[:, ob * 512 : (ob + 1) * 512], pso[:])
                        else:
                            nc.vector.tensor_copy(out_sb[:, ob * 512 : (ob + 1) * 512], pso[:])
                    nc.scalar.dma_start(
                        out=y_d[(sb * 4 + t) * P : (sb * 4 + t + 1) * P, :],
                        in_=out_sb[:],
                    )

    nc.compile()
    return nc


def host_prep(inputs):
    """Build per-core input arrays (layout + bf16 casts only, no math)."""
    x = np.asarray(inputs["x"], np.float32)

    gpk = np.zeros((P, GPK_COLS), np.float32)
    W1 = np.asarray(inputs["W1"], np.float32)
    gpk[:, C_W1T : C_W1T + 2 * CTR_HID] = (
        W1.T.reshape(2, P, CTR_HID).transpose(1, 0, 2).reshape(P, 2 * CTR_HID)
    )
    gpk[0:BS, C_CTR : C_CTR + CTR_OUT] = np.asarray(inputs["ctr_hidden_states"], np.float32)
    gpk[0:BS, C_GAM : C_GAM + CTR_OUT] = np.asarray(inputs["ln_gamma"], np.float32)[None, :]
    gpk[0:BS, C_BET : C_BET + CTR_OUT] = np.asarray(inputs["ln_beta"], np.float32)[None, :]
    gpk[0:CTR_HID, C_B1] = np.asarray(inputs["b1"], np.float32)
    gpk[0:CTR_HID, C_W2T : C_W2T + FD] = np.asarray(inputs["W2"], np.float32).T
    gpk[0:FD, C_B2] = np.asarray(inputs["b2"], np.float32)
    t16 = np.zeros((FD, P), np.float32)
    t16[np.arange(P) % FD, np.arange(P)] = 1.0
    gpk[0:FD, C_T16 : C_T16 + P] = t16
    mask = np.zeros((P, 2 * FD), np.float32)
    for r in range(FD):
        h, p0 = r // 8, (r % 8) * 16
        mask[p0 : p0 + FD, h * FD + r] = 1.0
    gpk[:, C_MSK : C_MSK + 2 * FD] = mask

    Wa = np.asarray(inputs["Wa"], np.float32)
    WaP = Wa.reshape(R, IN, FD).transpose(0, 2, 1).reshape(R * FD, IN)
    wap = np.ascontiguousarray(
        WaP.reshape(2, P, IN).transpose(1, 0, 2).reshape(P, 2 * IN)
    ).astype(BF)
    Wb = np.asarray(inputs["Wb"], np.float32) * SCALING
    WbP = Wb.reshape(R, OUT, FD).transpose(0, 2, 1).reshape(R * FD, OUT)
    wbp = np.ascontiguousarray(
        WbP.reshape(2, P, OUT).transpose(1, 0, 2).reshape(P, 2 * OUT)
    ).astype(BF)

    xbf = x.astype(BF)  # [8, 2048, 4096]

    in_maps = []
    for c in range(BS):
        g = gpk.copy()
        sel = np.zeros((FD, BS), np.float32)
        sel[:, c] = 1.0
        g[0:FD, C_SEL : C_SEL + BS] = sel
        # xtr[sb*128+p, cc*512+s] = x[c][sb*512+s, cc*128+p]
        xtr = np.ascontiguousarray(
            xbf[c].reshape(NSB, SBW, NC_I, P).transpose(0, 3, 2, 1)
        ).reshape(NSB * P, NC_I * SBW)
        in_maps.append({"gpk": g, "wap": wap, "wbp": wbp, "xtr": xtr})
    return in_maps


def get_compiled():
    global _COMPILED
    if _COMPILED is None:
        _COMPILED = build_program()
    return _COMPILED


def run(inputs, trace=False):
    from concourse.bass_utils import run_bass_kernel_spmd

    nc = get_compiled()
    in_maps = host_prep(inputs)
    res = run_bass_kernel_spmd(nc, in_maps, list(range(BS)), trace=trace)
    out = np.stack(
        [np.asarray(res.results[c]["y"]).astype(np.float32) for c in range(BS)], axis=0
    )
    return out, res


def kernel(**inputs) -> np.ndarray:
    out, _ = run(inputs, trace=False)
    return out


# revision 6
# speedup vs baseline: 2.1426x; 1.1536x over previous
"""Trainium2 Bass kernel for nn_Lorec (moe_routing LoRA-with-soft-routing).

Computation (per batch b):
  gate_b = softmax(MLP(LayerNorm(ctr[b])))                    [16]
  A_b[i,r] = sum_j Wa[r*4096+i, j] gate_b[j]                  [4096,16]
  B_b[r,o] = sum_j Wb[r*4096+o, j] gate_b[j]                  [16,4096]
  out[b] = (x[b] @ A_b) @ B_b * 2.0                           [2048,4096]

Sharding: data-parallel over bs=8 across 8 NeuronCores (one batch per core).
Gating replicated on every core; each core selects its own batch's gate row
via a per-core one-hot input.

v2 (bf16): all bulk tensors are bf16 (x, Wa, Wb, A, B, xa, y) — halves HBM
traffic vs f32 (75 MB -> ~37 MB per core) and rides the full-rate bf16 PE
path. x is pre-transposed and tiled on the host into [sb*128+p, c*512+s]
layout so mm1 needs no on-device transposes (lhsT = A chunk, rhs = xT tile).
Gating scalars packed into one [128, 1074] f32 tensor -> single DMA.
G (I_16 kron gate) built arithmetically: c16[p] = gate[p%16] via one tiny
matmul with a constant selector, then G = mask * c16 — no scatter DMAs.
Output written as bf16 and upcast on host. SCALING folded into Wb on host.
"""

import sys

sys.path.insert(0, "/opt/trn_rl_repo")

import numpy as np
import ml_dtypes

BF = ml_dtypes.bfloat16

BS = 8
SEQ = 2048
IN = 4096
OUT = 4096
R = 16
CTR_OUT = 256
CTR_HID = 60
FD = 16  # FINAL_DIM
LN_EPS = 1e-5
SCALING = 2.0

P = 128
NSB = 4  # s-blocks per core
SBW = 512  # s-block width
NC_I = IN // P  # 32 i-chunks
NG = 8  # x DMA groups per s-block (4 chunks = 512 KB each; keeps PE-wait < HAM window)
GCH = NC_I // NG  # chunks per group
NOB = OUT // 512  # 8 o-blocks
N_WARM = 16  # junk PE warmup matmuls at t=0 (HAM un-throttle)

# gpk column layout (packed f32 gating constants; partition rows as noted)
C_W1T = 0  # [128, 120]
C_CTR = 120  # [8, 256]
C_GAM = 376  # [8, 256]
C_BET = 632  # [8, 256]
C_B1 = 888  # [60, 1]
C_W2T = 889  # [60, 16]
C_B2 = 905  # [16, 1]
C_T16 = 906  # [16, 128]  t16[j, p] = (p % 16 == j)
C_MSK = 1034  # [128, 32]  mask[p, h*16+r] = (h == r//8 and p//16 == r%8)
C_SEL = 1066  # [16, 8]   per-core one-hot columns
GPK_COLS = 1074

_COMPILED = None


def build_program():
    import concourse.mybir as mybir
    from concourse import bacc
    from concourse.masks import make_identity
    from concourse.tile import TileContext

    f32 = mybir.dt.float32
    bf16 = mybir.dt.bfloat16
    AX = mybir.AxisListType.X
    ALU = mybir.AluOpType
    ACTF = mybir.ActivationFunctionType

    nc = bacc.Bacc("TRN2", target_bir_lowering=False, debug=False, num_devices=BS)

    xtr_d = nc.dram_tensor("xtr", [NSB * P, NC_I * SBW], bf16, kind="ExternalInput").ap()
    gpk_d = nc.dram_tensor("gpk", [P, GPK_COLS], f32, kind="ExternalInput").ap()
    wap_d = nc.dram_tensor("wap", [P, 2 * IN], bf16, kind="ExternalInput").ap()
    wbp_d = nc.dram_tensor("wbp", [P, 2 * OUT], bf16, kind="ExternalInput").ap()
    y_d = nc.dram_tensor("y", [SEQ, OUT], bf16, kind="ExternalOutput").ap()

    with TileContext(nc) as tc:
        with (
            tc.tile_pool(name="const", bufs=1) as const,
            tc.tile_pool(name="gp", bufs=1) as gp,
            tc.tile_pool(name="wstream", bufs=4) as wstream,
            tc.tile_pool(name="xpool", bufs=24) as xpool,
            tc.tile_pool(name="xapool", bufs=2) as xapool,
            tc.tile_pool(name="opool", bufs=3) as opool,
            tc.tile_pool(name="psg_pool", bufs=1, space="PSUM") as psg_pool,
            tc.tile_pool(name="psA_pool", bufs=1, space="PSUM") as psA_pool,
            tc.tile_pool(name="psB_pool", bufs=2, space="PSUM") as psB_pool,
            tc.tile_pool(name="psxa_pool", bufs=1, space="PSUM") as psxa_pool,
            tc.tile_pool(name="pso_pool", bufs=3, space="PSUM") as pso_pool,
        ):
            ident = const.tile([P, P], f32)
            make_identity(nc, ident)

            # ---- PE warm-up: junk matmuls while the first DMAs land, so the
            # HAM clock gate opens (1.2 -> 2.4 GHz) before real work arrives.
            for w in range(N_WARM):
                pwj = pso_pool.tile([P, 512], f32, tag="pso")
                nc.tensor.matmul(pwj[:, 0:P], ident[:], ident[:], start=True, stop=True)

            # ---- all HBM loads, queued in priority order on the sync ring ----
            gpk = gp.tile([P, GPK_COLS], f32)
            nc.sync.dma_start(out=gpk[:], in_=gpk_d[:])

            wa_t = []
            for h in range(2):
                wt = wstream.tile([P, IN], bf16, tag="wst")
                nc.sync.dma_start(out=wt[:], in_=wap_d[:, h * IN : (h + 1) * IN])
                wa_t.append(wt)
            wb_t = []
            for h in range(2):
                wt = wstream.tile([P, OUT], bf16, tag="wst")
                nc.sync.dma_start(out=wt[:], in_=wbp_d[:, h * OUT : (h + 1) * OUT])
                wb_t.append(wt)

            # ---- LayerNorm on ctr [8, 256] ----
            ctr = gpk[0:BS, C_CTR : C_CTR + CTR_OUT]
            gam = gpk[0:BS, C_GAM : C_GAM + CTR_OUT]
            bet = gpk[0:BS, C_BET : C_BET + CTR_OUT]
            mean = gp.tile([BS, 1], f32)
            xc = gp.tile([BS, CTR_OUT], f32)
            sq = gp.tile([BS, CTR_OUT], f32)
            vs = gp.tile([BS, 1], f32)
            std = gp.tile([BS, 1], f32)
            rstd = gp.tile([BS, 1], f32)
            hh = gp.tile([BS, CTR_OUT], f32)
            nc.vector.tensor_reduce(mean[:], ctr, axis=AX, op=ALU.add)
            nc.scalar.mul(mean[:], mean[:], 1.0 / CTR_OUT)
            nc.vector.tensor_scalar_sub(xc[:], ctr, mean[:])
            nc.vector.tensor_mul(sq[:], xc[:], xc[:])
            nc.vector.tensor_reduce(vs[:], sq[:], axis=AX, op=ALU.add)
            eps_t = gp.tile([BS, 1], f32)
            nc.gpsimd.memset(eps_t[:], LN_EPS)
            nc.scalar.activation(std[:], vs[:], ACTF.Sqrt, bias=eps_t[:], scale=1.0 / CTR_OUT)
            nc.vector.reciprocal(rstd[:], std[:])
            nc.vector.tensor_scalar_mul(hh[:], xc[:], rstd[:])
            nc.vector.tensor_mul(hh[:], hh[:], gam)
            nc.vector.tensor_add(hh[:], hh[:], bet)

            # ---- hT [256->2x128, 8] via PE transpose ----
            hT = gp.tile([P, 2 * BS], f32)
            for h in range(2):
                pt = psg_pool.tile([P, BS], f32, tag="psg")
                nc.tensor.transpose(pt[:], hh[:, h * P : (h + 1) * P], ident[0:BS, 0:BS])
                nc.scalar.copy(hT[:, h * BS : (h + 1) * BS], pt[:])

            # ---- h1T = relu(W1 @ h + b1) -> [60, 8] ----
            w1t = gpk[:, C_W1T : C_W1T + 2 * CTR_HID]
            ph1 = psg_pool.tile([CTR_HID, BS], f32, tag="psg")
            for h in range(2):
                nc.tensor.matmul(
                    ph1[:], w1t[:, h * CTR_HID : (h + 1) * CTR_HID],
                    hT[:, h * BS : (h + 1) * BS], start=(h == 0), stop=(h == 1),
                )
            h1T = gp.tile([CTR_HID, BS], f32)
            nc.scalar.activation(h1T[:], ph1[:], ACTF.Relu, bias=gpk[0:CTR_HID, C_B1 : C_B1 + 1])

            # ---- logitsT = W2 @ h1 + b2 -> [16, 8] ----
            plog = psg_pool.tile([FD, BS], f32, tag="psg")
            nc.tensor.matmul(plog[:], gpk[0:CTR_HID, C_W2T : C_W2T + FD], h1T[:], start=True, stop=True)
            logitsT = gp.tile([FD, BS], f32)
            nc.scalar.activation(logitsT[:], plog[:], ACTF.Identity, bias=gpk[0:FD, C_B2 : C_B2 + 1])

            # ---- softmax over FD per batch: transpose to [8, 16] ----
            plg = psg_pool.tile([BS, FD], f32, tag="psg")
            nc.tensor.transpose(plg[:], logitsT[:], ident[0:FD, 0:FD])
            lg = gp.tile([BS, FD], f32)
            nc.scalar.copy(lg[:], plg[:])
            mx = gp.tile([BS, 1], f32)
            ex = gp.tile([BS, FD], f32)
            sm = gp.tile([BS, 1], f32)
            rsm = gp.tile([BS, 1], f32)
            gate = gp.tile([BS, FD], f32)
            nc.vector.tensor_reduce(mx[:], lg[:], axis=AX, op=ALU.max)
            nc.vector.tensor_scalar_sub(ex[:], lg[:], mx[:])
            nc.scalar.activation(ex[:], ex[:], ACTF.Exp)
            nc.vector.tensor_reduce(sm[:], ex[:], axis=AX, op=ALU.add)
            nc.vector.reciprocal(rsm[:], sm[:])
            nc.vector.tensor_scalar_mul(gate[:], ex[:], rsm[:])

            # ---- gateT [16, 8], select own batch via one-hot columns ----
            pgT = psg_pool.tile([FD, BS], f32, tag="psg")
            nc.tensor.transpose(pgT[:], gate[:], ident[0:BS, 0:BS])
            gateT = gp.tile([FD, BS], f32)
            nc.scalar.copy(gateT[:], pgT[:])
            gsel = gp.tile([FD, BS], f32)
            gate_b = gp.tile([FD, 1], f32)
            nc.vector.tensor_mul(gsel[:], gateT[:], gpk[0:FD, C_SEL : C_SEL + BS])
            nc.vector.tensor_reduce(gate_b[:], gsel[:], axis=AX, op=ALU.add)

            # ---- G = I_16 kron gate_b, layout [128, 2*16], bf16 ----
            # c16[p] = gate_b[p % 16] via selector matmul, then G = mask * c16
            psc16 = psg_pool.tile([P, 1], f32, tag="psg")
            nc.tensor.matmul(
                psc16[:], gpk[0:FD, C_T16 : C_T16 + P], gate_b[:], start=True, stop=True
            )
            c16 = gp.tile([P, 1], f32)
            nc.scalar.copy(c16[:], psc16[:])
            Gf = gp.tile([P, 2 * FD], f32)
            nc.vector.tensor_scalar_mul(Gf[:], gpk[:, C_MSK : C_MSK + 2 * FD], c16[:])
            G = gp.tile([P, 2 * FD], bf16)
            nc.vector.tensor_copy(G[:], Gf[:])

            # ---- A-gen: A_sb[p, c*16+r] = A[c*128+p, r], bf16 ----
            A_sb = gp.tile([P, NC_I * R], bf16)
            psA = psA_pool.tile([P, 512], f32, tag="psA")
            for c in range(NC_I):
                for h in range(2):
                    nc.tensor.matmul(
                        psA[:, c * R : (c + 1) * R],
                        wa_t[h][:, c * P : (c + 1) * P],
                        G[:, h * FD : (h + 1) * FD],
                        start=(h == 0), stop=(h == 1),
                    )
            nc.scalar.copy(A_sb[:], psA[:])

            # ---- B-gen: B_sb [16, 4096] bf16 ----
            B_sb = gp.tile([FD, OUT], bf16)
            for ob in range(NOB):
                psB = psB_pool.tile([FD, 512], f32, tag="psB")
                for h in range(2):
                    nc.tensor.matmul(
                        psB[:],
                        G[:, h * FD : (h + 1) * FD],
                        wb_t[h][:, ob * 512 : (ob + 1) * 512],
                        start=(h == 0), stop=(h == 1),
                    )
                nc.vector.tensor_copy(B_sb[:, ob * 512 : (ob + 1) * 512], psB[:])

            # ---- main loop over s-blocks ----
            for sb in range(NSB):
                xts = []
                for g in range(NG):
                    xt = xpool.tile([P, GCH * SBW], bf16, tag="xg")
                    nc.sync.dma_start(
                        out=xt[:],
                        in_=xtr_d[sb * P : (sb + 1) * P, g * GCH * SBW : (g + 1) * GCH * SBW],
                    )
                    xts.append(xt)

                psxa = psxa_pool.tile([FD, SBW], f32, tag="psxa")
                for c in range(NC_I):
                    nc.tensor.matmul(
                        psxa[:],
                        A_sb[:, c * R : (c + 1) * R],
                        xts[c // GCH][:, (c % GCH) * SBW : (c % GCH + 1) * SBW],
                        start=(c == 0), stop=(c == NC_I - 1),
                    )
                xaT = xapool.tile([FD, SBW], bf16, tag="xaT")
                nc.scalar.copy(xaT[:], psxa[:])

                for t in range(4):
                    out_sb = opool.tile([P, OUT], bf16, tag="osb")
                    for ob in range(NOB):
                        pso = pso_pool.tile([P, 512], f32, tag="pso")
                        nc.tensor.matmul(
                            pso[:],
                            xaT[:, t * P : (t + 1) * P],
                            B_sb[:, ob * 512 : (ob + 1) * 512],
                            start=True, stop=True,
                        )
                        if ob % 2 == 0:
                            nc.scalar.copy(out_sb[:, ob * 512 : (ob + 1) * 512], pso[:])
                        else:
                            nc.vector.tensor_copy(out_sb[:, ob * 512 : (ob + 1) * 512], pso[:])
                    nc.scalar.dma_start(
                        out=y_d[(sb * 4 + t) * P : (sb * 4 + t + 1) * P, :],
                        in_=out_sb[:],
                    )

    nc.compile()
    return nc


def host_prep(inputs):
    """Build per-core input arrays (layout + bf16 casts only, no math)."""
    x = np.asarray(inputs["x"], np.float32)

    gpk = np.zeros((P, GPK_COLS), np.float32)
    W1 = np.asarray(inputs["W1"], np.float32)
    gpk[:, C_W1T : C_W1T + 2 * CTR_HID] = (
        W1.T.reshape(2, P, CTR_HID).transpose(1, 0, 2).reshape(P, 2 * CTR_HID)
    )
    gpk[0:BS, C_CTR : C_CTR + CTR_OUT] = np.asarray(inputs["ctr_hidden_states"], np.float32)
    gpk[0:BS, C_GAM : C_GAM + CTR_OUT] = np.asarray(inputs["ln_gamma"], np.float32)[None, :]
    gpk[0:BS, C_BET : C_BET + CTR_OUT] = np.asarray(inputs["ln_beta"], np.float32)[None, :]
    gpk[0:CTR_HID, C_B1] = np.asarray(inputs["b1"], np.float32)
    gpk[0:CTR_HID, C_W2T : C_W2T + FD] = np.asarray(inputs["W2"], np.float32).T
    gpk[0:FD, C_B2] = np.asarray(inputs["b2"], np.float32)
    t16 = np.zeros((FD, P), np.float32)
    t16[np.arange(P) % FD, np.arange(P)] = 1.0
    gpk[0:FD, C_T16 : C_T16 + P] = t16
    mask = np.zeros((P, 2 * FD), np.float32)
    for r in range(FD):
        h, p0 = r // 8, (r % 8) * 16
        mask[p0 : p0 + FD, h * FD + r] = 1.0
    gpk[:, C_MSK : C_MSK + 2 * FD] = mask

    Wa = np.asarray(inputs["Wa"], np.float32)
    WaP = Wa.reshape(R, IN, FD).transpose(0, 2, 1).reshape(R * FD, IN)
    wap = np.ascontiguousarray(
        WaP.reshape(2, P, IN).transpose(1, 0, 2).reshape(P, 2 * IN)
    ).astype(BF)
    Wb = np.asarray(inputs["Wb"], np.float32) * SCALING
    WbP = Wb.reshape(R, OUT, FD).transpose(0, 2, 1).reshape(R * FD, OUT)
    wbp = np.ascontiguousarray(
        WbP.reshape(2, P, OUT).transpose(1, 0, 2).reshape(P, 2 * OUT)
    ).astype(BF)

    xbf = x.astype(BF)  # [8, 2048, 4096]

    in_maps = []
    for c in range(BS):
        g = gpk.copy()
        sel = np.zeros((FD, BS), np.float32)
        sel[:, c] = 1.0
        g[0:FD, C_SEL : C_SEL + BS] = sel
        # xtr[sb*128+p, cc*512+s] = x[c][sb*512+s, cc*128+p]
        xtr = np.ascontiguousarray(
            xbf[c].reshape(NSB, SBW, NC_I, P).transpose(0, 3, 2, 1)
        ).reshape(NSB * P, NC_I * SBW)
        in_maps.append({"gpk": g, "wap": wap, "wbp": wbp, "xtr": xtr})
    return in_maps


def get_compiled():
    global _COMPILED
    if _COMPILED is None:
        _COMPILED = build_program()
    return _COMPILED


def run(inputs, trace=False):
    from concourse.bass_utils import run_bass_kernel_spmd

    nc = get_compiled()
    in_maps = host_prep(inputs)
    res = run_bass_kernel_spmd(nc, in_maps, list(range(BS)), trace=trace)
    out = np.stack(
        [np.asarray(res.results[c]["y"]).astype(np.float32) for c in range(BS)], axis=0
    )
    return out, res


def kernel(**inputs) -> np.ndarray:
    out, _ = run(inputs, trace=False)
    return out
